# revision 18
# baseline (speedup 1.0000x reference)
"""Trainium2 Bass kernel for OldNeighborhoodEncoder (segment_reduce).

Math (reference):
    fc1    = relu(X @ W1.T + b1)            # [N, 64], X = [N, 3]
    pooled = segment_max(fc1, cluster, S)   # [S, 64], cluster = arange(N)//32
    h      = relu(pooled @ W1g.T + b1g)     # [S, 64]
    out    = relu(h @ W2g.T + b2g)          # [S, 128]

Hardcoded sizes: N=1048576, S=32768 (32 pts/cluster), FEATURE=64, FG0=64,
FG1=128, 8 cores. Data-parallel over points: core d handles points
[d*131072, (d+1)*131072) == clusters [d*4096, (d+1)*4096); no collectives.

v2 design (vs v1's single-engine DVE reduce_max at 1 elem/cycle/lane):
  * bias b1 is folded into the fc1 matmul as an extra all-ones moving row
    (K=8: xyz+1 for two 512-point sets), so pooling max runs on pre-relu
    values and relu is applied once at the end (relu o max == max o relu).
  * matmul moving data xt is bf16 [16, 32768] in DRAM: two 8-row blocks at
    SBUF partition bases {0, 32} (base must be 0/32/64), matmul m=2w+j
    takes moving xt_t[32j:32j+8, 512w:512(w+1)].  16 DMA lanes vs v1's 6,
    and the 2.6x fewer input bytes make the input stream a non-issue.
  * PSUM can only be read by ACT and DVE (walrus: GPSIMD cannot access
    PSUM, DMA source must be SBUF/DRAM, and no instruction may read two
    PSUM operands), so pooling is split across exactly those two: 6 of
    every 8 chunks (A-route) are drained PSUM->SBUF fp16 by ACT
    (activation Relu, ~2.29us measured); the last 2 of each group
    (B-route) are drained by DVE with one reduce_max over the
    [128,4,16,4,8] view straight into the staged L3 slot (~2.75us), and
    are placed at group positions 6,7 so their psum frees immediately
    (the batched tree would otherwise sit in front of them in the DVE
    queue and stall the PE).
  * A-route tree is batched per group of 8 chunks to amortize the ~190ns
    DVE instruction overhead: drains land in s8[128,6,4,16,32], then one
    L1 (6144 charged elems, fp16 2x), one L2, one L3 into
    staged[128,8,4,16,4]; F1 4->2 (fp16 2x) and F2 2->1 fused with relu
    via scalar_tensor_tensor max(max(a,0),b) write pooled16.  The last
    group runs the tree per-chunk instead (3 small ops after each drain)
    so only ~3us of tree work trails the final drain.
  * tail MLP runs on fp16 pooled/h (PE rate for fp16 == f32r, fewer
    bytes), psum stays f32, biases f32; relu work split ACT/DVE as in v1.
"""

import sys
import numpy as np

if "/opt/trn_rl_repo" not in sys.path:
    sys.path.insert(0, "/opt/trn_rl_repo")

N = 1048576
S = 32768
PTS_PER_CLUSTER = 32
FEATURE = 64
FG0 = 64
FG1 = 128
NCORES = 8
NPC = N // NCORES          # 131072 points per core
SPC = S // NCORES          # 4096 clusters per core
NCHUNK = 32                # psum chunks per core (each = 4 matmuls of 512)

_PROGRAM = None  # (nc, input_names) cache


def _build_program():
    from concourse import bacc, bass, tile

    mybir = bass.mybir
    f32 = mybir.dt.float32
    bf16 = mybir.dt.bfloat16
    fp16 = mybir.dt.float16
    vmax = mybir.AluOpType.max
    vadd = mybir.AluOpType.add
    Relu = mybir.ActivationFunctionType.Relu
    AX = mybir.AxisListType

    nc = bacc.Bacc("TRN2", target_bir_lowering=False, debug=False)

    xtD = nc.dram_tensor("xtD", [16, 32768], bf16, kind="ExternalInput").ap()
    wrep = nc.dram_tensor("wrep", [128, 128], bf16, kind="ExternalInput").ap()
    w1gbd = nc.dram_tensor("w1gbd", [128, 128], fp16, kind="ExternalInput").ap()
    b1gd = nc.dram_tensor("b1gd", [128, 1], f32, kind="ExternalInput").ap()
    w2gt = nc.dram_tensor("w2gt", [128, 128], fp16, kind="ExternalInput").ap()
    b2g = nc.dram_tensor("b2g", [128, 1], f32, kind="ExternalInput").ap()
    outA = nc.dram_tensor("outA", [128, 2048], f32, kind="ExternalOutput").ap()
    outB = nc.dram_tensor("outB", [128, 2048], f32, kind="ExternalOutput").ap()

    with tile.TileContext(nc) as tc:
        with (
            tc.tile_pool(name="w", bufs=1) as wp,
            tc.tile_pool(name="x", bufs=1) as xp,
            tc.tile_pool(name="cv", bufs=2) as cvp,
            tc.tile_pool(name="t1", bufs=2) as t1p,
            tc.tile_pool(name="t2", bufs=2) as t2p,
            tc.tile_pool(name="st", bufs=2) as stp,
            tc.tile_pool(name="fin", bufs=2) as fip,
            tc.tile_pool(name="acc", bufs=1) as accp,
            tc.tile_pool(name="ps", bufs=2, space=bass.MemorySpace.PSUM) as pp,
        ):
            wrep_t = wp.tile([128, 128], bf16, tag="wrep")
            w1gbd_t = wp.tile([128, 128], fp16, tag="w1gbd")
            b1gd_t = wp.tile([128, 1], f32, tag="b1gd")
            w2gt_t = wp.tile([128, 128], fp16, tag="w2gt")
            b2g_t = wp.tile([128, 1], f32, tag="b2g")
            # weight DMAs on the Scalar queue (HWDGE); wrep first — it
            # gates the first matmul.
            for t, d in (
                (wrep_t, wrep),
                (w1gbd_t, w1gbd),
                (b1gd_t, b1gd),
                (w2gt_t, w2gt),
                (b2g_t, b2g),
            ):
                nc.scalar.dma_start(t[:], d[:])

            # input stream: one [128, 32768] bf16 tile, rows 0:8 and 32:40
            # used.  Pieces sized in half-chunks (512 cols), small first so
            # the opening matmul isn't gated on a bulk transfer.
            xt_t = xp.tile([128, 32768], bf16, tag="xt")
            for pi, (c0, nhc) in enumerate(
                ((0, 1), (1, 1), (2, 2), (4, 6), (10, 8), (18, 14),
                 (32, 16), (48, 16))
            ):
                cs = slice(512 * c0, 512 * (c0 + nhc))
                nc.sync.dma_start(xt_t[0:8, cs], xtD[0:8, cs])
                # piece 0's second block rides the scalar queue (right
                # after the tiny weight DMAs) so both blocks land sooner
                q = nc.scalar if pi == 0 else nc.sync
                q.dma_start(xt_t[32:40, cs], xtD[8:16, cs])

            # pooled16[p, v, kk, b, q]: pooled fc1 (post-relu) for cluster
            # 128*(8v+kk) + 32b + 16a + q, feature f, where p = 64a + f.
            pooled16 = accp.tile([128, 4, 8, 4, 16], fp16, tag="pooled16")

            # Per group: 6 A-chunks (i=0..5, ACT-drained into s8) then 2
            # B-chunks (i=6,7, DVE reduce).  L1 is split in two (slots 0:3
            # after drain 2, slots 3:6 after drain 5) so DVE has ready
            # work while it would otherwise idle waiting for the B-chunks'
            # matmuls; the B-reduces still precede L2b in the queue so
            # their psum tiles free fast.  Tail layer-2 for group g-2 is
            # dropped into the PE stream after chunk 7's matmuls (its F2'
            # completed a whole group ago, so the PE never blocks on it).
            hR = accp.tile([128, 2048], fp16, tag="hR")
            for g in range(4):
                last = g == 3
                # g3 flips its B-chunks to the front and runs the last two
                # trees per-chunk, so only ~3us of tree work trails the
                # final drain instead of the whole batched chain.
                bset = (0, 1) if last else (6, 7)
                soff = 2 if last else 0  # chunk i -> s8 slot i - soff
                s8 = cvp.tile([128, 6, 4, 16, 32], fp16, tag="s8")
                stg_t = stp.tile([128, 8, 4, 16, 4], fp16, tag="stg")
                t1b = t1p.tile([128, 6, 4, 16, 16], fp16, tag="t1b")
                for i in range(8):
                    k = 8 * g + i
                    ps = pp.tile([128, 4, 16, 32], f32, tag="ps")
                    for b in range(4):
                        m = 4 * k + b
                        j, w = m % 2, m // 2
                        nc.tensor.matmul(
                            ps[:, b],
                            wrep_t[32 * j : 32 * j + 8, :],
                            xt_t[32 * j : 32 * j + 8, 512 * w : 512 * (w + 1)],
                        )
                    if i not in bset:
                        # A-route: ACT drains psum -> fp16 with relu fused
                        # (relu before max is fine: max is monotone, and
                        # the final relu at F2 is idempotent on these).
                        slot = i - soff
                        nc.scalar.activation(s8[:, slot], ps[:], Relu)
                        if not last:
                            if i == 2:
                                nc.vector.tensor_tensor(
                                    t1b[:, 0:3], s8[:, 0:3, :, :, 0:16],
                                    s8[:, 0:3, :, :, 16:32], vmax
                                )
                            elif i == 5:
                                nc.vector.tensor_tensor(
                                    t1b[:, 3:6], s8[:, 3:6, :, :, 0:16],
                                    s8[:, 3:6, :, :, 16:32], vmax
                                )
                        elif i == 5:
                            # g3: batch chunks 2-5 (slots 0-3) mid-group
                            t2b4 = t2p.tile([128, 6, 4, 16, 8], fp16, tag="t2b")
                            nc.vector.tensor_tensor(
                                t1b[:, 0:4], s8[:, 0:4, :, :, 0:16],
                                s8[:, 0:4, :, :, 16:32], vmax
                            )
                            nc.vector.tensor_tensor(
                                t2b4[:, 0:4], t1b[:, 0:4, :, :, 0:8],
                                t1b[:, 0:4, :, :, 8:16], vmax
                            )
                            nc.vector.tensor_tensor(
                                stg_t[:, 2:6], t2b4[:, 0:4, :, :, 0:4],
                                t2b4[:, 0:4, :, :, 4:8], vmax
                            )
                        elif i >= 6:
                            # g3: per-chunk tree for chunks 6,7
                            t1 = t1p.tile([128, 4, 16, 16], fp16, tag="t1")
                            nc.vector.tensor_tensor(
                                t1[:], s8[:, slot, :, :, 0:16],
                                s8[:, slot, :, :, 16:32], vmax
                            )
                            t2 = t2p.tile([128, 4, 16, 8], fp16, tag="t2")
                            nc.vector.tensor_tensor(
                                t2[:], t1[:, :, :, 0:8], t1[:, :, :, 8:16], vmax
                            )
                            nc.vector.tensor_tensor(
                                stg_t[:, i], t2[:, :, :, 0:4], t2[:, :, :, 4:8],
                                vmax
                            )
                    else:
                        # B-route: DVE drains psum with one 8-wide
                        # reduce_max (pre-relu values; F2's fused relu
                        # fixes them up).
                        nc.vector.reduce_max(
                            stg_t[:, i],
                            ps[:].rearrange("p a b (c d) -> p a b c d", c=4),
                            axis=AX.X,
                        )
                        if i == 7 and g == 2:
                            hpsx = pp.tile([128, 4, 16, 32], f32, tag="ps")
                            nc.tensor.matmul(
                                hpsx[:, 0],
                                w1gbd_t[:],
                                pooled16[:, 0].rearrange("p a b c -> p (a b c)"),
                            )
                            nc.scalar.activation(
                                hR[:, 0:512], hpsx[:, 0], Relu, bias=b1gd_t[:]
                            )
                if not last:
                    t2b = t2p.tile([128, 6, 4, 16, 8], fp16, tag="t2b")
                    nc.vector.tensor_tensor(
                        t2b[:], t1b[:, :, :, :, 0:8], t1b[:, :, :, :, 8:16], vmax
                    )
                    nc.vector.tensor_tensor(
                        stg_t[:, 0:6], t2b[:, :, :, :, 0:4], t2b[:, :, :, :, 4:8],
                        vmax
                    )
                    if g == 2:
                        hpsy = pp.tile([128, 4, 16, 32], f32, tag="ps")
                        nc.tensor.matmul(
                            hpsy[:, 0],
                            w1gbd_t[:],
                            pooled16[:, 1].rearrange("p a b c -> p (a b c)"),
                        )
                        nc.scalar.activation(
                            hR[:, 512:1024], hpsy[:, 0], Relu, bias=b1gd_t[:]
                        )
                fin = fip.tile([128, 8, 4, 16, 2], fp16, tag="fin")
                nc.vector.tensor_tensor(
                    fin[:], stg_t[:, :, :, :, 0:2], stg_t[:, :, :, :, 2:4], vmax
                )
                # F2 + relu: max(max(a, 0), b)
                nc.vector.scalar_tensor_tensor(
                    pooled16[:, g],
                    fin[:, :, :, :, 0],
                    0.0,
                    fin[:, :, :, :, 1],
                    op0=vmax,
                    op1=vmax,
                )

            # remaining tail: layer-2 for g2 (ready now) and g3 (waits
            # F2'(3) -- the PE has nothing else left by then), layer-3 in
            # 512-col slices with o2A relu on DVE / o2B on ACT, and
            # quarter-split output DMAs as each slice lands.  psum banks
            # are packed [hps2|A0|B0|-], [A1|B1|A2|B2], [hps3|A3|B3|-] so
            # no tile allocation ever waits on work emitted after it.
            o2A = accp.tile([128, 2048], f32, tag="o2A")
            o2B = accp.tile([128, 2048], f32, tag="o2B")
            tp1 = pp.tile([128, 4, 16, 32], f32, tag="ps")
            tp2 = pp.tile([128, 4, 16, 32], f32, tag="ps")

            def l3_slice(j, pa, pb):
                nc.vector.tensor_scalar(
                    o2A[:, j * 512 : (j + 1) * 512],
                    pa, b2g_t[:], 0.0, op0=vadd, op1=vmax,
                )
                nc.sync.dma_start(
                    outA[:, j * 512 : (j + 1) * 512],
                    o2A[:, j * 512 : (j + 1) * 512],
                )
                nc.scalar.activation(
                    o2B[:, j * 512 : (j + 1) * 512], pb, Relu, bias=b2g_t[:]
                )
                nc.scalar.dma_start(
                    outB[:, j * 512 : (j + 1) * 512],
                    o2B[:, j * 512 : (j + 1) * 512],
                )

            nc.tensor.matmul(
                tp1[:, 0],
                w1gbd_t[:],
                pooled16[:, 2].rearrange("p a b c -> p (a b c)"),
            )
            nc.scalar.activation(
                hR[:, 1024:1536], tp1[:, 0], Relu, bias=b1gd_t[:]
            )
            nc.tensor.matmul(tp1[:, 1], w2gt_t[0:64, :], hR[0:64, 0:512])
            nc.tensor.matmul(tp1[:, 2], w2gt_t[64:128, :], hR[64:128, 0:512])
            l3_slice(0, tp1[:, 1], tp1[:, 2])
            nc.tensor.matmul(tp2[:, 0], w2gt_t[0:64, :], hR[0:64, 512:1024])
            nc.tensor.matmul(tp2[:, 1], w2gt_t[64:128, :], hR[64:128, 512:1024])
            l3_slice(1, tp2[:, 0], tp2[:, 1])
            nc.tensor.matmul(tp2[:, 2], w2gt_t[0:64, :], hR[0:64, 1024:1536])
            nc.tensor.matmul(tp2[:, 3], w2gt_t[64:128, :], hR[64:128, 1024:1536])
            l3_slice(2, tp2[:, 2], tp2[:, 3])
            tp3 = pp.tile([128, 4, 16, 32], f32, tag="ps")
            nc.tensor.matmul(
                tp3[:, 0],
                w1gbd_t[:],
                pooled16[:, 3].rearrange("p a b c -> p (a b c)"),
            )
            nc.scalar.activation(
                hR[:, 1536:2048], tp3[:, 0], Relu, bias=b1gd_t[:]
            )
            nc.tensor.matmul(tp3[:, 1], w2gt_t[0:64, :], hR[0:64, 1536:2048])
            nc.tensor.matmul(tp3[:, 2], w2gt_t[64:128, :], hR[64:128, 1536:2048])
            l3_slice(3, tp3[:, 1], tp3[:, 2])

    nc.compile()
    return nc


def _get_program():
    global _PROGRAM
    if _PROGRAM is None:
        _PROGRAM = _build_program()
    return _PROGRAM


def _host_pack(relative_points, W1, b1, W1g, b1g, W2g, b2g):
    import ml_dtypes

    bf16 = ml_dtypes.bfloat16
    X = np.ascontiguousarray(relative_points, dtype=np.float32)
    W1 = np.asarray(W1, np.float32)
    b1 = np.asarray(b1, np.float32)
    W1g = np.asarray(W1g, np.float32)
    b1g = np.asarray(b1g, np.float32)
    W2g = np.asarray(W2g, np.float32)
    b2g = np.asarray(b2g, np.float32)

    # stationary block: rows 0-2 W1.T -> outs 0:64, row 3 b1; rows 4-7 the
    # same for outs 64:128.  Replicated at partition bases 0 and 32.
    blk = np.zeros((8, 128), np.float32)
    blk[0:3, 0:64] = W1.T
    blk[3, 0:64] = b1
    blk[4:7, 64:128] = W1.T
    blk[7, 64:128] = b1
    wrep = np.zeros((128, 128), np.float32)
    wrep[0:8] = blk
    wrep[32:40] = blk
    wrep = wrep.astype(bf16)

    w1gbd = np.zeros((128, 128), np.float32)
    w1gbd[0:64, 0:64] = W1g.T
    w1gbd[64:128, 64:128] = W1g.T
    w1gbd = w1gbd.astype(np.float16)
    b1gd = np.concatenate([b1g, b1g]).reshape(128, 1)
    w2gt = np.vstack([W2g.T, W2g.T]).astype(np.float16)  # [128, 128]
    b2gc = np.ascontiguousarray(b2g.reshape(128, 1))

    in_maps = []
    for d in range(NCORES):
        Xc = X[d * NPC : (d + 1) * NPC]
        # xt8[4h+r, m, o]: r=0..2 xyz of point 1024m+512h+o, r=3 ones
        t = Xc.reshape(128, 2, 512, 3).transpose(1, 3, 0, 2)  # [h,xyz,m,o]
        xt8 = np.empty((2, 4, 128, 512), np.float32)
        xt8[:, 0:3] = t
        xt8[:, 3] = 1.0
        # xtD[8j+r, 512w+o] = xt8[r, m=2w+j, o]
        xtD = np.ascontiguousarray(
            xt8.reshape(8, 64, 2, 512).transpose(2, 0, 1, 3).reshape(16, 32768)
        ).astype(bf16)
        in_maps.append(
            {
                "xtD": xtD,
                "wrep": wrep,
                "w1gbd": w1gbd,
                "b1gd": b1gd,
                "w2gt": w2gt,
                "b2g": b2gc,
            }
        )
    return in_maps


def _host_unpack(results):
    out = np.empty((S, FG1), np.float32)
    for d in range(NCORES):
        oA = results[d]["outA"].reshape(128, NCHUNK, 4, 16)
        oB = results[d]["outB"].reshape(128, NCHUNK, 4, 16)
        blk = out[d * SPC : (d + 1) * SPC].reshape(NCHUNK, 4, 2, 16, 128)
        blk[:, :, 0] = oA.transpose(1, 2, 3, 0)
        blk[:, :, 1] = oB.transpose(1, 2, 3, 0)
    return out


def _numpy_fallback(relative_points, cluster, num_clusters,
                    W1, b1, W1g, b1g, W2g, b2g):
    X = np.asarray(relative_points, np.float32)
    fc1 = np.maximum(X @ np.asarray(W1, np.float32).T + np.asarray(b1, np.float32), 0.0)
    Sn = int(num_clusters)
    cl = np.asarray(cluster).astype(np.int64)
    pooled = np.full((Sn, fc1.shape[1]), -np.inf, np.float32)
    # sorted segment ids -> reduceat over run starts
    starts = np.flatnonzero(np.r_[True, cl[1:] != cl[:-1]])
    seg_ids = cl[starts]
    pooled[seg_ids] = np.maximum.reduceat(fc1, starts, axis=0)
    h = np.maximum(pooled @ np.asarray(W1g, np.float32).T + np.asarray(b1g, np.float32), 0.0)
    return np.maximum(h @ np.asarray(W2g, np.float32).T + np.asarray(b2g, np.float32), 0.0).astype(np.float32)


def _run_hw(in_maps, trace=False):
    from concourse.bass_utils import run_bass_kernel_spmd

    nc = _get_program()
    return run_bass_kernel_spmd(
        nc, in_maps, list(range(NCORES)), trace=trace
    )


def kernel(relative_points, cluster, num_clusters,
           W1, b1, W1g, b1g, W2g, b2g):
    cl = np.asarray(cluster)
    expected_cl = np.arange(N, dtype=np.int64) // PTS_PER_CLUSTER
    if (
        relative_points.shape != (N, 3)
        or int(num_clusters) != S
        or not np.array_equal(cl, expected_cl)
    ):
        return _numpy_fallback(relative_points, cluster, num_clusters,
                               W1, b1, W1g, b1g, W2g, b2g)

    in_maps = _host_pack(relative_points, W1, b1, W1g, b1g, W2g, b2g)
    res = _run_hw(in_maps, trace=False)
    return _host_unpack(res.results)


def run_traced(inputs):
    """test.py helper: returns (output, exec_time_ns)."""
    in_maps = _host_pack(
        inputs["relative_points"], inputs["W1"], inputs["b1"],
        inputs["W1g"], inputs["b1g"], inputs["W2g"], inputs["b2g"],
    )
    res = _run_hw(in_maps, trace=True)
    return _host_unpack(res.results), res.exec_time_ns


# revision 20
# speedup vs baseline: 1.0462x; 1.0462x over previous
"""Trainium2 Bass kernel for OldNeighborhoodEncoder (segment_reduce).

Math (reference):
    fc1    = relu(X @ W1.T + b1)            # [N, 64], X = [N, 3]
    pooled = segment_max(fc1, cluster, S)   # [S, 64], cluster = arange(N)//32
    h      = relu(pooled @ W1g.T + b1g)     # [S, 64]
    out    = relu(h @ W2g.T + b2g)          # [S, 128]

Hardcoded sizes: N=1048576, S=32768 (32 pts/cluster), FEATURE=64, FG0=64,
FG1=128, 8 cores. Data-parallel over points: core d handles points
[d*131072, (d+1)*131072) == clusters [d*4096, (d+1)*4096); no collectives.

v2 design (vs v1's single-engine DVE reduce_max at 1 elem/cycle/lane):
  * bias b1 is folded into the fc1 matmul as an extra all-ones moving row
    (K=8: xyz+1 for two 512-point sets), so pooling max runs on pre-relu
    values and relu is applied once at the end (relu o max == max o relu).
  * matmul moving data xt is bf16 [16, 32768] in DRAM: two 8-row blocks at
    SBUF partition bases {0, 32} (base must be 0/32/64), matmul m=2w+j
    takes moving xt_t[32j:32j+8, 512w:512(w+1)].  16 DMA lanes vs v1's 6,
    and the 2.6x fewer input bytes make the input stream a non-issue.
  * PSUM can only be read by ACT and DVE (walrus: GPSIMD cannot access
    PSUM, DMA source must be SBUF/DRAM, and no instruction may read two
    PSUM operands), so pooling is split across exactly those two: 6 of
    every 8 chunks (A-route) are drained PSUM->SBUF fp16 by ACT
    (activation Relu, ~2.29us measured); the last 2 of each group
    (B-route) are drained by DVE with one reduce_max over the
    [128,4,16,4,8] view straight into the staged L3 slot (~2.75us), and
    are placed at group positions 6,7 so their psum frees immediately
    (the batched tree would otherwise sit in front of them in the DVE
    queue and stall the PE).
  * A-route tree is batched per group of 8 chunks to amortize the ~190ns
    DVE instruction overhead: drains land in s8[128,6,4,16,32], then one
    L1 (6144 charged elems, fp16 2x), one L2, one L3 into
    staged[128,8,4,16,4]; F1 4->2 (fp16 2x) and F2 2->1 fused with relu
    via scalar_tensor_tensor max(max(a,0),b) write pooled16.  The last
    group runs the tree per-chunk instead (3 small ops after each drain)
    so only ~3us of tree work trails the final drain.
  * tail MLP runs on fp16 pooled/h (PE rate for fp16 == f32r, fewer
    bytes), psum stays f32, biases f32; relu work split ACT/DVE as in v1.
"""

import sys
import numpy as np

if "/opt/trn_rl_repo" not in sys.path:
    sys.path.insert(0, "/opt/trn_rl_repo")

N = 1048576
S = 32768
PTS_PER_CLUSTER = 32
FEATURE = 64
FG0 = 64
FG1 = 128
NCORES = 8
NPC = N // NCORES          # 131072 points per core
SPC = S // NCORES          # 4096 clusters per core
NCHUNK = 32                # psum chunks per core (each = 4 matmuls of 512)

_PROGRAM = None  # (nc, input_names) cache


def _build_program():
    from concourse import bacc, bass, tile

    mybir = bass.mybir
    f32 = mybir.dt.float32
    bf16 = mybir.dt.bfloat16
    fp16 = mybir.dt.float16
    vmax = mybir.AluOpType.max
    vadd = mybir.AluOpType.add
    Relu = mybir.ActivationFunctionType.Relu
    AX = mybir.AxisListType

    nc = bacc.Bacc("TRN2", target_bir_lowering=False, debug=False)

    xtD = nc.dram_tensor("xtD", [16, 32768], bf16, kind="ExternalInput").ap()
    wrep = nc.dram_tensor("wrep", [128, 128], bf16, kind="ExternalInput").ap()
    w1gbd = nc.dram_tensor("w1gbd", [128, 128], fp16, kind="ExternalInput").ap()
    b1gd = nc.dram_tensor("b1gd", [128, 1], f32, kind="ExternalInput").ap()
    w2gt = nc.dram_tensor("w2gt", [128, 128], fp16, kind="ExternalInput").ap()
    b2g = nc.dram_tensor("b2g", [128, 1], f32, kind="ExternalInput").ap()
    outA = nc.dram_tensor("outA", [128, 2048], f32, kind="ExternalOutput").ap()
    outB = nc.dram_tensor("outB", [128, 2048], f32, kind="ExternalOutput").ap()

    with tile.TileContext(nc) as tc:
        with (
            tc.tile_pool(name="w", bufs=1) as wp,
            tc.tile_pool(name="x", bufs=1) as xp,
            tc.tile_pool(name="cv", bufs=2) as cvp,
            tc.tile_pool(name="t1", bufs=2) as t1p,
            tc.tile_pool(name="t2", bufs=2) as t2p,
            tc.tile_pool(name="st", bufs=2) as stp,
            tc.tile_pool(name="fin", bufs=2) as fip,
            tc.tile_pool(name="acc", bufs=1) as accp,
            tc.tile_pool(name="ps", bufs=2, space=bass.MemorySpace.PSUM) as pp,
        ):
            wrep_t = wp.tile([128, 128], bf16, tag="wrep")
            w1gbd_t = wp.tile([128, 128], fp16, tag="w1gbd")
            b1gd_t = wp.tile([128, 1], f32, tag="b1gd")
            w2gt_t = wp.tile([128, 128], fp16, tag="w2gt")
            b2g_t = wp.tile([128, 1], f32, tag="b2g")
            # weight DMAs on the Scalar queue (HWDGE); wrep first — it
            # gates the first matmul.
            for t, d in (
                (wrep_t, wrep),
                (w1gbd_t, w1gbd),
                (b1gd_t, b1gd),
                (w2gt_t, w2gt),
                (b2g_t, b2g),
            ):
                nc.scalar.dma_start(t[:], d[:])

            # input stream: one [128, 32768] bf16 tile, rows 0:8 and 32:40
            # used.  Pieces sized in half-chunks (512 cols), small first so
            # the opening matmul isn't gated on a bulk transfer.
            xt_t = xp.tile([128, 32768], bf16, tag="xt")
            for c0, nhc in ((0, 1), (1, 1), (2, 2), (4, 6), (10, 8), (18, 14),
                            (32, 16), (48, 16)):
                cs = slice(512 * c0, 512 * (c0 + nhc))
                nc.sync.dma_start(xt_t[0:8, cs], xtD[0:8, cs])
                nc.sync.dma_start(xt_t[32:40, cs], xtD[8:16, cs])

            # pooled16[p, v, kk, b, q]: pooled fc1 (post-relu) for cluster
            # 128*(8v+kk) + 32b + 16a + q, feature f, where p = 64a + f.
            pooled16 = accp.tile([128, 4, 8, 4, 16], fp16, tag="pooled16")

            # Per group: 6 A-chunks (i=0..5, ACT-drained into s8) then 2
            # B-chunks (i=6,7, DVE reduce).  L1 is split in two (slots 0:3
            # after drain 2, slots 3:6 after drain 5) so DVE has ready
            # work while it would otherwise idle waiting for the B-chunks'
            # matmuls; the B-reduces still precede L2b in the queue so
            # their psum tiles free fast.  Tail layer-2 for group g-2 is
            # dropped into the PE stream after chunk 7's matmuls (its F2'
            # completed a whole group ago, so the PE never blocks on it).
            hR = accp.tile([128, 2048], fp16, tag="hR")
            for g in range(4):
                s8 = cvp.tile([128, 6, 4, 16, 32], fp16, tag="s8")
                stg_t = stp.tile([128, 8, 4, 16, 4], fp16, tag="stg")
                t1b = t1p.tile([128, 6, 4, 16, 16], fp16, tag="t1b")
                for i in range(8):
                    k = 8 * g + i
                    ps = pp.tile([128, 4, 16, 32], f32, tag="ps")
                    for b in range(4):
                        m = 4 * k + b
                        j, w = m % 2, m // 2
                        nc.tensor.matmul(
                            ps[:, b],
                            wrep_t[32 * j : 32 * j + 8, :],
                            xt_t[32 * j : 32 * j + 8, 512 * w : 512 * (w + 1)],
                        )
                    if i < 6:
                        # A-route: ACT drains psum -> fp16 with relu fused
                        # (relu before max is fine: max is monotone, and
                        # the final relu at F2 is idempotent on these).
                        nc.scalar.activation(s8[:, i], ps[:], Relu)
                        if i == 2:
                            nc.vector.tensor_tensor(
                                t1b[:, 0:3], s8[:, 0:3, :, :, 0:16],
                                s8[:, 0:3, :, :, 16:32], vmax
                            )
                        elif i == 5:
                            nc.vector.tensor_tensor(
                                t1b[:, 3:6], s8[:, 3:6, :, :, 0:16],
                                s8[:, 3:6, :, :, 16:32], vmax
                            )
                    else:
                        # B-route: DVE drains psum with one 8-wide
                        # reduce_max (pre-relu values; F2's fused relu
                        # fixes them up).
                        nc.vector.reduce_max(
                            stg_t[:, i],
                            ps[:].rearrange("p a b (c d) -> p a b c d", c=4),
                            axis=AX.X,
                        )
                        if i == 7 and g >= 2:
                            gp = g - 2
                            hpsx = pp.tile([128, 4, 16, 32], f32, tag="ps")
                            nc.tensor.matmul(
                                hpsx[:, 0],
                                w1gbd_t[:],
                                pooled16[:, gp].rearrange("p a b c -> p (a b c)"),
                            )
                            nc.scalar.activation(
                                hR[:, gp * 512 : (gp + 1) * 512],
                                hpsx[:, 0],
                                Relu,
                                bias=b1gd_t[:],
                            )
                t2b = t2p.tile([128, 6, 4, 16, 8], fp16, tag="t2b")
                nc.vector.tensor_tensor(
                    t2b[:], t1b[:, :, :, :, 0:8], t1b[:, :, :, :, 8:16], vmax
                )
                nc.vector.tensor_tensor(
                    stg_t[:, 0:6], t2b[:, :, :, :, 0:4], t2b[:, :, :, :, 4:8],
                    vmax
                )
                fin = fip.tile([128, 8, 4, 16, 2], fp16, tag="fin")
                nc.vector.tensor_tensor(
                    fin[:], stg_t[:, :, :, :, 0:2], stg_t[:, :, :, :, 2:4], vmax
                )
                # F2 + relu: max(max(a, 0), b)
                nc.vector.scalar_tensor_tensor(
                    pooled16[:, g],
                    fin[:, :, :, :, 0],
                    0.0,
                    fin[:, :, :, :, 1],
                    op0=vmax,
                    op1=vmax,
                )

            # remaining tail: layer-2 for g2 (ready now) and g3 (waits
            # F2'(3) -- the PE has nothing else left by then), layer-3 in
            # 512-col slices with o2A relu on DVE / o2B on ACT, and
            # quarter-split output DMAs as each slice lands.  psum banks
            # are packed [hps2|A0|B0|-], [A1|B1|A2|B2], [hps3|A3|B3|-] so
            # no tile allocation ever waits on work emitted after it.
            o2A = accp.tile([128, 2048], f32, tag="o2A")
            o2B = accp.tile([128, 2048], f32, tag="o2B")
            tp1 = pp.tile([128, 4, 16, 32], f32, tag="ps")
            tp2 = pp.tile([128, 4, 16, 32], f32, tag="ps")

            def l3_slice(j, pa, pb):
                nc.vector.tensor_scalar(
                    o2A[:, j * 512 : (j + 1) * 512],
                    pa, b2g_t[:], 0.0, op0=vadd, op1=vmax,
                )
                nc.sync.dma_start(
                    outA[:, j * 512 : (j + 1) * 512],
                    o2A[:, j * 512 : (j + 1) * 512],
                )
                nc.scalar.activation(
                    o2B[:, j * 512 : (j + 1) * 512], pb, Relu, bias=b2g_t[:]
                )
                nc.scalar.dma_start(
                    outB[:, j * 512 : (j + 1) * 512],
                    o2B[:, j * 512 : (j + 1) * 512],
                )

            nc.tensor.matmul(
                tp1[:, 0],
                w1gbd_t[:],
                pooled16[:, 2].rearrange("p a b c -> p (a b c)"),
            )
            nc.scalar.activation(
                hR[:, 1024:1536], tp1[:, 0], Relu, bias=b1gd_t[:]
            )
            nc.tensor.matmul(tp1[:, 1], w2gt_t[0:64, :], hR[0:64, 0:512])
            nc.tensor.matmul(tp1[:, 2], w2gt_t[64:128, :], hR[64:128, 0:512])
            l3_slice(0, tp1[:, 1], tp1[:, 2])
            nc.tensor.matmul(tp2[:, 0], w2gt_t[0:64, :], hR[0:64, 512:1024])
            nc.tensor.matmul(tp2[:, 1], w2gt_t[64:128, :], hR[64:128, 512:1024])
            l3_slice(1, tp2[:, 0], tp2[:, 1])
            nc.tensor.matmul(tp2[:, 2], w2gt_t[0:64, :], hR[0:64, 1024:1536])
            nc.tensor.matmul(tp2[:, 3], w2gt_t[64:128, :], hR[64:128, 1024:1536])
            l3_slice(2, tp2[:, 2], tp2[:, 3])
            tp3 = pp.tile([128, 4, 16, 32], f32, tag="ps")
            nc.tensor.matmul(
                tp3[:, 0],
                w1gbd_t[:],
                pooled16[:, 3].rearrange("p a b c -> p (a b c)"),
            )
            nc.scalar.activation(
                hR[:, 1536:2048], tp3[:, 0], Relu, bias=b1gd_t[:]
            )
            nc.tensor.matmul(tp3[:, 1], w2gt_t[0:64, :], hR[0:64, 1536:2048])
            nc.tensor.matmul(tp3[:, 2], w2gt_t[64:128, :], hR[64:128, 1536:2048])
            l3_slice(3, tp3[:, 1], tp3[:, 2])

    nc.compile()
    return nc


def _get_program():
    global _PROGRAM
    if _PROGRAM is None:
        _PROGRAM = _build_program()
    return _PROGRAM


def _host_pack(relative_points, W1, b1, W1g, b1g, W2g, b2g):
    import ml_dtypes

    bf16 = ml_dtypes.bfloat16
    X = np.ascontiguousarray(relative_points, dtype=np.float32)
    W1 = np.asarray(W1, np.float32)
    b1 = np.asarray(b1, np.float32)
    W1g = np.asarray(W1g, np.float32)
    b1g = np.asarray(b1g, np.float32)
    W2g = np.asarray(W2g, np.float32)
    b2g = np.asarray(b2g, np.float32)

    # stationary block: rows 0-2 W1.T -> outs 0:64, row 3 b1; rows 4-7 the
    # same for outs 64:128.  Replicated at partition bases 0 and 32.
    blk = np.zeros((8, 128), np.float32)
    blk[0:3, 0:64] = W1.T
    blk[3, 0:64] = b1
    blk[4:7, 64:128] = W1.T
    blk[7, 64:128] = b1
    wrep = np.zeros((128, 128), np.float32)
    wrep[0:8] = blk
    wrep[32:40] = blk
    wrep = wrep.astype(bf16)

    w1gbd = np.zeros((128, 128), np.float32)
    w1gbd[0:64, 0:64] = W1g.T
    w1gbd[64:128, 64:128] = W1g.T
    w1gbd = w1gbd.astype(np.float16)
    b1gd = np.concatenate([b1g, b1g]).reshape(128, 1)
    w2gt = np.vstack([W2g.T, W2g.T]).astype(np.float16)  # [128, 128]
    b2gc = np.ascontiguousarray(b2g.reshape(128, 1))

    in_maps = []
    for d in range(NCORES):
        Xc = X[d * NPC : (d + 1) * NPC]
        # xt8[4h+r, m, o]: r=0..2 xyz of point 1024m+512h+o, r=3 ones
        t = Xc.reshape(128, 2, 512, 3).transpose(1, 3, 0, 2)  # [h,xyz,m,o]
        xt8 = np.empty((2, 4, 128, 512), np.float32)
        xt8[:, 0:3] = t
        xt8[:, 3] = 1.0
        # xtD[8j+r, 512w+o] = xt8[r, m=2w+j, o]
        xtD = np.ascontiguousarray(
            xt8.reshape(8, 64, 2, 512).transpose(2, 0, 1, 3).reshape(16, 32768)
        ).astype(bf16)
        in_maps.append(
            {
                "xtD": xtD,
                "wrep": wrep,
                "w1gbd": w1gbd,
                "b1gd": b1gd,
                "w2gt": w2gt,
                "b2g": b2gc,
            }
        )
    return in_maps


def _host_unpack(results):
    out = np.empty((S, FG1), np.float32)
    for d in range(NCORES):
        oA = results[d]["outA"].reshape(128, NCHUNK, 4, 16)
        oB = results[d]["outB"].reshape(128, NCHUNK, 4, 16)
        blk = out[d * SPC : (d + 1) * SPC].reshape(NCHUNK, 4, 2, 16, 128)
        blk[:, :, 0] = oA.transpose(1, 2, 3, 0)
        blk[:, :, 1] = oB.transpose(1, 2, 3, 0)
    return out


def _numpy_fallback(relative_points, cluster, num_clusters,
                    W1, b1, W1g, b1g, W2g, b2g):
    X = np.asarray(relative_points, np.float32)
    fc1 = np.maximum(X @ np.asarray(W1, np.float32).T + np.asarray(b1, np.float32), 0.0)
    Sn = int(num_clusters)
    cl = np.asarray(cluster).astype(np.int64)
    pooled = np.full((Sn, fc1.shape[1]), -np.inf, np.float32)
    # sorted segment ids -> reduceat over run starts
    starts = np.flatnonzero(np.r_[True, cl[1:] != cl[:-1]])
    seg_ids = cl[starts]
    pooled[seg_ids] = np.maximum.reduceat(fc1, starts, axis=0)
    h = np.maximum(pooled @ np.asarray(W1g, np.float32).T + np.asarray(b1g, np.float32), 0.0)
    return np.maximum(h @ np.asarray(W2g, np.float32).T + np.asarray(b2g, np.float32), 0.0).astype(np.float32)


def _run_hw(in_maps, trace=False):
    from concourse.bass_utils import run_bass_kernel_spmd

    nc = _get_program()
    return run_bass_kernel_spmd(
        nc, in_maps, list(range(NCORES)), trace=trace
    )


def kernel(relative_points, cluster, num_clusters,
           W1, b1, W1g, b1g, W2g, b2g):
    cl = np.asarray(cluster)
    expected_cl = np.arange(N, dtype=np.int64) // PTS_PER_CLUSTER
    if (
        relative_points.shape != (N, 3)
        or int(num_clusters) != S
        or not np.array_equal(cl, expected_cl)
    ):
        return _numpy_fallback(relative_points, cluster, num_clusters,
                               W1, b1, W1g, b1g, W2g, b2g)

    in_maps = _host_pack(relative_points, W1, b1, W1g, b1g, W2g, b2g)
    res = _run_hw(in_maps, trace=False)
    return _host_unpack(res.results)


def run_traced(inputs):
    """test.py helper: returns (output, exec_time_ns)."""
    in_maps = _host_pack(
        inputs["relative_points"], inputs["W1"], inputs["b1"],
        inputs["W1g"], inputs["b1g"], inputs["W2g"], inputs["b2g"],
    )
    res = _run_hw(in_maps, trace=True)
    return _host_unpack(res.results), res.exec_time_ns
